# revision 20
# baseline (speedup 1.0000x reference)
"""DeltaRuleUpdate kernel for 8x Trainium2 NeuronCores (Bass/Tile).

Math (per b,h):
    phi  = ELU(K) + 1                 (S, DK)
    G    = phi^T @ phi                (DK, DK)
    U    = phi^T @ V                  (DK, DV)
    zinc = phi^T @ ones               (DK,)
    new_M = M + U - G @ M
    new_z = z + zinc

No (S,DV) intermediate is materialized -> the kernel streams K,V once
(memory-bound).  PSUM accumulation over s is order-agnostic, so K/V
chunks are DMA'd *flat*: each partition holds CH consecutive s-rows as
one contiguous span (large DMA descriptors, near line-rate).  The
matmuls then contract over partitions; subtile j covers rows
{chunk_base + p*CH + j : p in 0..127}.

phi is computed as relu(K) + min(exp(K), 1) == ELU(K)+1 (no native ELU
table): exp on ScalarE, relu + fused min/add on VectorE.

MODE 'fp32' (exact): two accumulating fp32 PE matmuls per s-subtile
    psum_Gz += phi_j^T @ [phi_j | 1]   (N=129)
    psum_U  += phi_j^T @ V_j           (N=128)
MODE 'f32r' (tf32 operands, fp32 PSUM accumulate): one fused matmul
    psum    += phi_j^T @ [phi_j | V_j | 1 | 1]   (N=258 even, 4x faster rows;
    the trailing ones column doubles as finite padding - fp32r matmuls
    require an even moving dim)

Sharding: B*H = 64 pairs split 8-per-core across 8 cores; fully
data-parallel, no collectives.
"""

import sys

if "/opt/trn_rl_repo" not in sys.path:
    sys.path.insert(0, "/opt/trn_rl_repo")

import numpy as np

B, H, S, DK, DV = 4, 16, 8192, 128, 128
N_CORES = 8
PAIRS = (B * H) // N_CORES  # pairs per core
P = 128                     # partitions
MODE = "fp16"               # 'fp32' | 'f32r' | 'fp16'
CH = 16                     # s-subtiles per DMA chunk (chunk = CH*128 rows)
NCH = S // (P * CH)         # chunks per pair
PHI_W = 136                 # fp32-mode phi subtile stride (phi 128 | one | pad)
COMB_W = 264                # f32r-mode subtile stride (phi 128 | one | V 128 | pad)
RHS_W = DK + DV + 2         # 258: [phi | V | 1 | 1]
CHUNK_EL = CH * P * DK      # elements per K/V chunk

_built = {}


def _build_nc():
    key = (CH, MODE)
    if key in _built:
        return _built[key]

    import concourse.bass as bass  # noqa: F401
    import concourse.tile as tile
    from concourse import bacc, mybir

    fp32 = mybir.dt.float32
    f32r = mybir.dt.float32r
    fp16 = mybir.dt.float16
    kv_dt = fp16 if MODE == "fp16" else fp32
    mm_dt = fp16 if MODE == "fp16" else f32r
    Alu = mybir.AluOpType
    Act = mybir.ActivationFunctionType

    nc = bacc.Bacc("TRN2", target_bir_lowering=False, debug=False,
                   num_devices=N_CORES)

    KV_d = nc.dram_tensor("KV", [PAIRS, NCH, 2, P, CH * DK], kv_dt,
                          kind="ExternalInput").ap()
    M_d = nc.dram_tensor("M", [PAIRS, DK, DV], fp32, kind="ExternalInput").ap()
    z_d = nc.dram_tensor("z", [PAIRS, DK], fp32, kind="ExternalInput").ap()
    MO_d = nc.dram_tensor("MO", [PAIRS, DK, DV], fp32, kind="ExternalOutput").ap()
    ZO_d = nc.dram_tensor("ZO", [PAIRS, DK], fp32, kind="ExternalOutput").ap()

    with tile.TileContext(nc) as tc:
        with (
            tc.tile_pool(name="onesp", bufs=1) as onesp,
            tc.tile_pool(name="kp", bufs=5) as kp,
            tc.tile_pool(name="vp", bufs=5) as vp,
            tc.tile_pool(name="pp", bufs=3) as pp,
            tc.tile_pool(name="rp", bufs=2) as rp,
            tc.tile_pool(name="mp", bufs=2) as mp,
            tc.tile_pool(name="zp", bufs=2) as zp,
            tc.tile_pool(name="gp", bufs=2) as gp,
            tc.tile_pool(name="op", bufs=2) as op,
            tc.tile_pool(name="ozp", bufs=2) as ozp,
            tc.tile_pool(name="pgz", bufs=4, space="PSUM") as pgz,
            tc.tile_pool(name="pu", bufs=2, space="PSUM") as pu,
            tc.tile_pool(name="pgm", bufs=2, space="PSUM") as pgm,
        ):
            ones_t = onesp.tile([P, 1], kv_dt)
            nc.gpsimd.memset(ones_t[:], 1.0)
            for pair in range(PAIRS):
                m_t = mp.tile([DK, DV], fp32)
                nc.gpsimd.dma_start(out=m_t[:], in_=M_d[pair])
                z_t = zp.tile([DK, 1], fp32)
                nc.gpsimd.dma_start(out=z_t[:], in_=z_d[pair].unsqueeze(1))

                if MODE == "fp32":
                    gz = pgz.tile([P, DK + 1], fp32)
                    u = pu.tile([P, DV], fp32)
                else:
                    acc = pgz.tile([P, RHS_W], fp32)

                items = [(c, 0, CH) for c in range(NCH)]
                if pair == PAIRS - 1:
                    h = CH // 2
                    items = items[:-1] + [(NCH - 1, 0, h), (NCH - 1, h, h)]
                mm_i = 0
                n_mm = NCH * CH
                for (c, off, nsub) in items:
                    k_t = kp.tile([P, nsub, DK], kv_dt, tag="k_t")
                    r_t = rp.tile([P, nsub, DK], kv_dt, tag="r_t")

                    nc.sync.dma_start(
                        out=k_t[:],
                        in_=KV_d[pair, c, 0][:, off * DK:(off + nsub) * DK]
                        .rearrange("p (n d) -> p n d", d=DK))

                    if MODE == "fp32":
                        v_t = vp.tile([P, CH, DK], kv_dt)
                        nc.sync.dma_start(
                            out=v_t[:],
                            in_=KV_d[pair, c, 1].rearrange(
                                "p (n d) -> p n d", d=DK))
                        p_t = pp.tile([P, CH, PHI_W], fp32)
                        nc.gpsimd.memset(p_t[:, :, DK:DK + 1], 1.0)
                        nc.scalar.activation(p_t[:, :, 0:DK], k_t[:], Act.Exp)
                        nc.vector.tensor_scalar_max(r_t[:], k_t[:], 0.0)
                        nc.vector.scalar_tensor_tensor(
                            p_t[:, :, 0:DK], p_t[:, :, 0:DK], 1.0, r_t[:],
                            Alu.min, Alu.add)
                        for j in range(CH):
                            i = c * CH + j
                            first, last = (i == 0), (i == NCH * CH - 1)
                            lhsT = p_t[:, j, 0:DK]
                            nc.tensor.matmul(gz[:], lhsT, p_t[:, j, 0:DK + 1],
                                             start=first, stop=last)
                            nc.tensor.matmul(u[:], lhsT, v_t[:, j, :],
                                             start=first, stop=last)
                    else:
                        comb = pp.tile([P, nsub, COMB_W], mm_dt, tag="comb")
                        e_t = rp.tile([P, nsub, DK], fp32, tag="e_t")
                        v_t = vp.tile([P, nsub, DK], kv_dt, tag="v_t")
                        nc.sync.dma_start(
                            out=v_t[:],
                            in_=KV_d[pair, c, 1][:, off * DK:(off + nsub) * DK]
                            .rearrange("p (n d) -> p n d", d=DK))
                        nc.vector.tensor_copy(
                            comb[:, :, DK + DV:DK + DV + 2],
                            ones_t[:].unsqueeze(1).broadcast_to([P, nsub, 2]))
                        nc.scalar.activation(e_t[:], k_t[:], Act.Exp)
                        nc.vector.tensor_scalar_max(r_t[:], k_t[:], 0.0)
                        # phi = min(exp,1) + relu
                        nc.vector.scalar_tensor_tensor(
                            comb[:, :, 0:DK], e_t[:], 1.0, r_t[:],
                            Alu.min, Alu.add)
                        # V -> comb, split across ScalarE and VectorE
                        nc.scalar.copy(
                            comb[:, :, DK:DK + DV // 2],
                            v_t[:, :, 0:DV // 2])
                        nc.vector.tensor_copy(
                            comb[:, :, DK + DV // 2:DK + DV],
                            v_t[:, :, DV // 2:DV])
                        for j in range(nsub):
                            first, last = (mm_i == 0), (mm_i == n_mm - 1)
                            mm_i += 1
                            nc.tensor.matmul(
                                acc[:], comb[:, j, 0:DK],
                                comb[:, j, 0:RHS_W],
                                start=first, stop=last)

                # epilogue: new_M = M + U - G@M ; new_z = z + zinc
                if MODE == "fp32":
                    g_src, z_src, u_src = gz[:, 0:DK], gz[:, DK:DK + 1], u[:]
                else:
                    g_src = acc[:, 0:DK]
                    u_src = acc[:, DK:DK + DV]
                    z_src = acc[:, DK + DV:DK + DV + 1]
                g_t = gp.tile([DK, DK], fp32)
                nc.scalar.copy(g_t[:], g_src)
                gm = pgm.tile([DK, DV], fp32)
                nc.tensor.matmul(gm[:], g_t[:], m_t[:], start=True, stop=True)
                newm = op.tile([DK, DV], fp32)
                nc.vector.tensor_add(newm[:], m_t[:], u_src)
                nc.vector.tensor_sub(newm[:], newm[:], gm[:])
                out_eng = nc.sync if pair == PAIRS - 1 else nc.gpsimd
                out_eng.dma_start(out=MO_d[pair], in_=newm[:])
                newz = ozp.tile([DK, 1], fp32)
                nc.vector.tensor_add(newz[:], z_t[:], z_src)
                out_eng.dma_start(out=ZO_d[pair].unsqueeze(1), in_=newz[:])

    nc.compile()
    _built[key] = nc
    return nc


def make_in_maps(K, V, M, z):
    kv_np = np.float16 if MODE == "fp16" else np.float32
    Kf = np.asarray(K, dtype=kv_np).reshape(B * H, NCH, P, CH * DK)
    Vf = np.asarray(V, dtype=kv_np).reshape(B * H, NCH, P, CH * DK)
    KVf = np.stack([Kf, Vf], axis=2)  # (B*H, NCH, 2, P, CH*DK)
    Mf = np.ascontiguousarray(np.asarray(M, dtype=np.float32)).reshape(B * H, DK, DV)
    zf = np.ascontiguousarray(np.asarray(z, dtype=np.float32)).reshape(B * H, DK)

    in_maps = []
    for c in range(N_CORES):
        sl = slice(c * PAIRS, (c + 1) * PAIRS)
        in_maps.append({"KV": np.ascontiguousarray(KVf[sl]),
                        "M": Mf[sl], "z": zf[sl]})
    return in_maps


def kernel(K, V, M, z):
    from concourse.bass_utils import run_bass_kernel_spmd

    nc = _build_nc()
    in_maps = make_in_maps(K, V, M, z)
    res = run_bass_kernel_spmd(nc, in_maps, core_ids=list(range(N_CORES)))

    new_M = np.concatenate(
        [res.results[c]["MO"] for c in range(N_CORES)], axis=0
    ).reshape(B, H, DK, DV)
    new_z = np.concatenate(
        [res.results[c]["ZO"] for c in range(N_CORES)], axis=0
    ).reshape(B, H, DK)
    return new_M, new_z


# revision 21
# speedup vs baseline: 1.1232x; 1.1232x over previous
"""DeltaRuleUpdate kernel for 8x Trainium2 NeuronCores (Bass/Tile).

Math (per b,h):
    phi  = ELU(K) + 1                 (S, DK)
    G    = phi^T @ phi                (DK, DK)
    U    = phi^T @ V                  (DK, DV)
    zinc = phi^T @ ones               (DK,)
    new_M = M + U - G @ M
    new_z = z + zinc

No (S,DV) intermediate is materialized -> the kernel streams K,V once
(memory-bound).  PSUM accumulation over s is order-agnostic, so K/V
chunks are DMA'd *flat*: each partition holds CH consecutive s-rows as
one contiguous span (large DMA descriptors, near line-rate).  The
matmuls then contract over partitions; subtile j covers rows
{chunk_base + p*CH + j : p in 0..127}.

phi is computed as relu(K) + min(exp(K), 1) == ELU(K)+1 (no native ELU
table): exp on ScalarE, relu + fused min/add on VectorE.

MODE 'fp32' (exact): two accumulating fp32 PE matmuls per s-subtile
    psum_Gz += phi_j^T @ [phi_j | 1]   (N=129)
    psum_U  += phi_j^T @ V_j           (N=128)
MODE 'f32r' (tf32 operands, fp32 PSUM accumulate): one fused matmul
    psum    += phi_j^T @ [phi_j | V_j | 1 | 1]   (N=258 even, 4x faster rows;
    the trailing ones column doubles as finite padding - fp32r matmuls
    require an even moving dim)

Sharding: B*H = 64 pairs split 8-per-core across 8 cores; fully
data-parallel, no collectives.
"""

import sys

if "/opt/trn_rl_repo" not in sys.path:
    sys.path.insert(0, "/opt/trn_rl_repo")

import numpy as np

B, H, S, DK, DV = 4, 16, 8192, 128, 128
N_CORES = 8
PAIRS = (B * H) // N_CORES  # pairs per core
P = 128                     # partitions
MODE = "fp16"               # 'fp32' | 'f32r' | 'fp16'
CH = 32                     # s-subtiles per DMA chunk (chunk = CH*128 rows)
NCH = S // (P * CH)         # chunks per pair
PHI_W = 136                 # fp32-mode phi subtile stride (phi 128 | one | pad)
COMB_W = 264                # f32r-mode subtile stride (phi 128 | one | V 128 | pad)
RHS_W = DK + DV + 2         # 258: [phi | V | 1 | 1]
CHUNK_EL = CH * P * DK      # elements per K/V chunk

_built = {}


def _build_nc():
    key = (CH, MODE)
    if key in _built:
        return _built[key]

    import concourse.bass as bass  # noqa: F401
    import concourse.tile as tile
    from concourse import bacc, mybir

    fp32 = mybir.dt.float32
    f32r = mybir.dt.float32r
    fp16 = mybir.dt.float16
    kv_dt = fp16 if MODE == "fp16" else fp32
    mm_dt = fp16 if MODE == "fp16" else f32r
    Alu = mybir.AluOpType
    Act = mybir.ActivationFunctionType

    nc = bacc.Bacc("TRN2", target_bir_lowering=False, debug=False,
                   num_devices=N_CORES)

    KV_d = nc.dram_tensor("KV", [PAIRS, NCH, 2, P, CH * DK], kv_dt,
                          kind="ExternalInput").ap()
    M_d = nc.dram_tensor("M", [PAIRS, DK, DV], fp32, kind="ExternalInput").ap()
    z_d = nc.dram_tensor("z", [PAIRS, DK], fp32, kind="ExternalInput").ap()
    MO_d = nc.dram_tensor("MO", [PAIRS, DK, DV], fp32, kind="ExternalOutput").ap()
    ZO_d = nc.dram_tensor("ZO", [PAIRS, DK], fp32, kind="ExternalOutput").ap()

    with tile.TileContext(nc) as tc:
        with (
            tc.tile_pool(name="onesp", bufs=1) as onesp,
            tc.tile_pool(name="kp", bufs=4) as kp,
            tc.tile_pool(name="vp", bufs=4) as vp,
            tc.tile_pool(name="pp", bufs=3) as pp,
            tc.tile_pool(name="rp", bufs=2) as rp,
            tc.tile_pool(name="mp", bufs=2) as mp,
            tc.tile_pool(name="zp", bufs=2) as zp,
            tc.tile_pool(name="gp", bufs=2) as gp,
            tc.tile_pool(name="op", bufs=2) as op,
            tc.tile_pool(name="ozp", bufs=2) as ozp,
            tc.tile_pool(name="pgz", bufs=4, space="PSUM") as pgz,
            tc.tile_pool(name="pu", bufs=2, space="PSUM") as pu,
            tc.tile_pool(name="pgm", bufs=2, space="PSUM") as pgm,
        ):
            ones_t = onesp.tile([P, 1], kv_dt)
            nc.gpsimd.memset(ones_t[:], 1.0)
            for pair in range(PAIRS):
                m_t = mp.tile([DK, DV], fp32)
                nc.gpsimd.dma_start(out=m_t[:], in_=M_d[pair])
                z_t = zp.tile([DK, 1], fp32)
                nc.gpsimd.dma_start(out=z_t[:], in_=z_d[pair].unsqueeze(1))

                if MODE == "fp32":
                    gz = pgz.tile([P, DK + 1], fp32)
                    u = pu.tile([P, DV], fp32)
                else:
                    acc = pgz.tile([P, RHS_W], fp32)

                items = [(c, 0, CH) for c in range(NCH)]
                if pair == PAIRS - 1:
                    h, q = CH // 2, CH // 4
                    items = items[:-1] + [(NCH - 1, 0, h),
                                          (NCH - 1, h, q),
                                          (NCH - 1, h + q, q)]
                mm_i = 0
                n_mm = NCH * CH
                for (c, off, nsub) in items:
                    k_t = kp.tile([P, nsub, DK], kv_dt, tag="k_t")
                    r_t = rp.tile([P, nsub, DK], kv_dt, tag="r_t")

                    nc.sync.dma_start(
                        out=k_t[:],
                        in_=KV_d[pair, c, 0][:, off * DK:(off + nsub) * DK]
                        .rearrange("p (n d) -> p n d", d=DK))

                    if MODE == "fp32":
                        v_t = vp.tile([P, CH, DK], kv_dt)
                        nc.sync.dma_start(
                            out=v_t[:],
                            in_=KV_d[pair, c, 1].rearrange(
                                "p (n d) -> p n d", d=DK))
                        p_t = pp.tile([P, CH, PHI_W], fp32)
                        nc.gpsimd.memset(p_t[:, :, DK:DK + 1], 1.0)
                        nc.scalar.activation(p_t[:, :, 0:DK], k_t[:], Act.Exp)
                        nc.vector.tensor_scalar_max(r_t[:], k_t[:], 0.0)
                        nc.vector.scalar_tensor_tensor(
                            p_t[:, :, 0:DK], p_t[:, :, 0:DK], 1.0, r_t[:],
                            Alu.min, Alu.add)
                        for j in range(CH):
                            i = c * CH + j
                            first, last = (i == 0), (i == NCH * CH - 1)
                            lhsT = p_t[:, j, 0:DK]
                            nc.tensor.matmul(gz[:], lhsT, p_t[:, j, 0:DK + 1],
                                             start=first, stop=last)
                            nc.tensor.matmul(u[:], lhsT, v_t[:, j, :],
                                             start=first, stop=last)
                    else:
                        comb = pp.tile([P, nsub, COMB_W], mm_dt, tag="comb")
                        e_t = rp.tile([P, nsub, DK], fp32, tag="e_t")
                        v_t = vp.tile([P, nsub, DK], kv_dt, tag="v_t")
                        nc.sync.dma_start(
                            out=v_t[:],
                            in_=KV_d[pair, c, 1][:, off * DK:(off + nsub) * DK]
                            .rearrange("p (n d) -> p n d", d=DK))
                        nc.vector.tensor_copy(
                            comb[:, :, DK + DV:DK + DV + 2],
                            ones_t[:].unsqueeze(1).broadcast_to([P, nsub, 2]))
                        nc.scalar.activation(e_t[:], k_t[:], Act.Exp)
                        nc.vector.tensor_scalar_max(r_t[:], k_t[:], 0.0)
                        # phi = min(exp,1) + relu
                        nc.vector.scalar_tensor_tensor(
                            comb[:, :, 0:DK], e_t[:], 1.0, r_t[:],
                            Alu.min, Alu.add)
                        # V -> comb, split across ScalarE and VectorE
                        nc.scalar.copy(
                            comb[:, :, DK:DK + DV // 2],
                            v_t[:, :, 0:DV // 2])
                        nc.vector.tensor_copy(
                            comb[:, :, DK + DV // 2:DK + DV],
                            v_t[:, :, DV // 2:DV])
                        for j in range(nsub):
                            first, last = (mm_i == 0), (mm_i == n_mm - 1)
                            mm_i += 1
                            nc.tensor.matmul(
                                acc[:], comb[:, j, 0:DK],
                                comb[:, j, 0:RHS_W],
                                start=first, stop=last)

                # epilogue: new_M = M + U - G@M ; new_z = z + zinc
                if MODE == "fp32":
                    g_src, z_src, u_src = gz[:, 0:DK], gz[:, DK:DK + 1], u[:]
                else:
                    g_src = acc[:, 0:DK]
                    u_src = acc[:, DK:DK + DV]
                    z_src = acc[:, DK + DV:DK + DV + 1]
                g_t = gp.tile([DK, DK], fp32)
                nc.scalar.copy(g_t[:], g_src)
                gm = pgm.tile([DK, DV], fp32)
                nc.tensor.matmul(gm[:], g_t[:], m_t[:], start=True, stop=True)
                newm = op.tile([DK, DV], fp32)
                nc.vector.tensor_add(newm[:], m_t[:], u_src)
                nc.vector.tensor_sub(newm[:], newm[:], gm[:])
                out_eng = nc.sync if pair == PAIRS - 1 else nc.gpsimd
                out_eng.dma_start(out=MO_d[pair], in_=newm[:])
                newz = ozp.tile([DK, 1], fp32)
                nc.vector.tensor_add(newz[:], z_t[:], z_src)
                out_eng.dma_start(out=ZO_d[pair].unsqueeze(1), in_=newz[:])

    nc.compile()
    _built[key] = nc
    return nc


def make_in_maps(K, V, M, z):
    kv_np = np.float16 if MODE == "fp16" else np.float32
    Kf = np.asarray(K, dtype=kv_np).reshape(B * H, NCH, P, CH * DK)
    Vf = np.asarray(V, dtype=kv_np).reshape(B * H, NCH, P, CH * DK)
    KVf = np.stack([Kf, Vf], axis=2)  # (B*H, NCH, 2, P, CH*DK)
    Mf = np.ascontiguousarray(np.asarray(M, dtype=np.float32)).reshape(B * H, DK, DV)
    zf = np.ascontiguousarray(np.asarray(z, dtype=np.float32)).reshape(B * H, DK)

    in_maps = []
    for c in range(N_CORES):
        sl = slice(c * PAIRS, (c + 1) * PAIRS)
        in_maps.append({"KV": np.ascontiguousarray(KVf[sl]),
                        "M": Mf[sl], "z": zf[sl]})
    return in_maps


def kernel(K, V, M, z):
    from concourse.bass_utils import run_bass_kernel_spmd

    nc = _build_nc()
    in_maps = make_in_maps(K, V, M, z)
    res = run_bass_kernel_spmd(nc, in_maps, core_ids=list(range(N_CORES)))

    new_M = np.concatenate(
        [res.results[c]["MO"] for c in range(N_CORES)], axis=0
    ).reshape(B, H, DK, DV)
    new_z = np.concatenate(
        [res.results[c]["ZO"] for c in range(N_CORES)], axis=0
    ).reshape(B, H, DK)
    return new_M, new_z
